# revision 36
# baseline (speedup 1.0000x reference)
"""Adaptive-softmax NLL loss on 8 TRN2 NeuronCores.

Strategy: tensor-parallel over the vocab dimension, fp8(e4m3) DoubleRow
matmuls (2 fp8 weights per PE cell -> 256-deep contraction per pass).
Each core computes the exp-sums of its vocab slice of head / tail1 /
tail2 logits for all 4096 tokens, plus (token-sharded) the gathered
target-logit dot products. One AllReduce combines per-token sum-exp
partials; every core then finishes the scalar NLL identically.

NLL = sum_n log(S_head_n) + sum_{c1} log(S_t1_n) + sum_{c2} log(S_t2_n)
      - sum_n x_n . W_ext[cidx_n] - sum_{c1} h1_n . W1[t_n-C0] - sum_{c2} h2_n . W2[t_n-C1]

Weights are scaled by 64 on the host before the fp8 cast (keeps values
out of the e4m3 subnormal range); the exp/copy activations undo the
scale with scale=1/64. Logit exps are written to SBUF in bf16 and
summed on the vector engine (2x packed mode), keeping the scalar
engine (the throughput limiter) on pure exp work.
"""

import os
import sys

for _p in ("/opt/trn_rl_repo",):
    if _p not in sys.path:
        sys.path.insert(0, _p)

import ml_dtypes
import numpy as np

import concourse.bacc as bacc
import concourse.bass as bass
import concourse.bass_isa as bass_isa
import concourse.mybir as mybir
import concourse.tile as tile
from concourse.bass_utils import run_bass_kernel_spmd

dt = mybir.dt
AF = mybir.ActivationFunctionType
ALU = mybir.AluOpType
DR = mybir.MatmulPerfMode.DoubleRow
AXX = mybir.AxisListType.X

NCORES = 8
N, D = 4096, 1024
C0, C1, C2 = 20000, 40000, 50257
VH = C0 + 2          # head logits incl 2 cluster columns
R1, R2 = 256, 64
VHC = 2504           # head vocab cols per core (8*2504 = 20032, pad 30)
V1C = 2500           # tail1 cols per core     (8*2500 = 20000, pad 0)
V2C = 1283           # tail2 cols per core     (8*1283 = 10264, pad 7)
PAD_H = NCORES * VHC - VH
PAD_2 = NCORES * V2C - (C2 - C1)
NT = N // 128        # 32 token tiles
NSH = N // NCORES    # 512 tokens per core for the sharded head dot
WS = 64.0            # host-side weight scale before fp8 cast
INV = 1.0 / WS

F32, BF16, F8 = dt.float32, dt.bfloat16, dt.float8e4
NP_F8 = ml_dtypes.float8_e4m3

# phase-C units: (cluster, base col, width); each <= 1024 (2 PSUM banks)
UNITS = [("H", 0, 1024), ("H", 1024, 1024), ("H", 2048, VHC - 2048),
         ("T1", 0, 1024), ("T1", 1024, 1024), ("T1", 2048, V1C - 2048),
         ("T2", 0, 1024), ("T2", 1024, V2C - 1024)]
NU = len(UNITS)
NS = 6              # sum slots per tile: H01, H2, T101, T12, T20, T21
ART = 24            # tiles covered by the early AllReduce chunk
PAYA = 3 * ART + 1  # early payload: H/T1/T2 sums for tiles 0..ART-1 + dsh
PAYB = 3 * (NT - ART)

LAST_EXEC_NS = None


# XOR-butterfly remote-DMA tail reduction: logic is in place but the
# axon/fake-NRT environment does not implement the SWDGE remote rings
# (HW deadlocks; even MultiCoreSim needs a real NC mapping), so the
# chunked collective AllReduce path is the default.
BFLY = bool(int(os.environ.get("KERNEL_BFLY", "0")))


def _build():
    nc = bacc.Bacc("TRN2", target_bir_lowering=False, debug=False,
                   num_devices=NCORES)

    U8 = dt.uint8
    xq_in = nc.declare_dram_parameter("xq", [128, 8 * N], U8, isOutput=False)
    whq_in = nc.declare_dram_parameter("whq", [128, 8 * VHC], U8, isOutput=False)
    w1q_in = nc.declare_dram_parameter("w1q", [128, 2 * V1C], U8, isOutput=False)
    w2q_in = nc.declare_dram_parameter("w2q", [64, V2C], U8, isOutput=False)
    p1q_in = nc.declare_dram_parameter("p1q", [128, 8 * R1], U8, isOutput=False)
    p2q_in = nc.declare_dram_parameter("p2q", [128, 8 * R2], U8, isOutput=False)
    w1sq_in = nc.declare_dram_parameter("w1sq", [128, 2 * N], U8, isOutput=False)
    w2sq_in = nc.declare_dram_parameter("w2sq", [64, N], U8, isOutput=False)
    xTc_in = nc.declare_dram_parameter("xTc", [D, NSH], F32, isOutput=False)
    wselT_in = nc.declare_dram_parameter("wselT", [D, NSH], F32, isOutput=False)
    m1_in = nc.declare_dram_parameter("m1", [128, NT], F32, isOutput=False)
    m2_in = nc.declare_dram_parameter("m2", [128, NT], F32, isOutput=False)
    out_ext = nc.declare_dram_parameter("out", [1, 1], F32, isOutput=True)

    # Buffers shared with the raw post-tile block (physical SBUF addresses —
    # the ISA-class remote-DMA instructions there cannot take symbolic
    # tile-pool APs). Allocated before the TileContext so the two
    # allocators don't collide.
    pay_b = nc.alloc_sbuf_tensor("pay_b", [128, PAYB], F32)
    red_a = nc.alloc_sbuf_tensor("red_a", [128, PAYA], F32)
    tot_a = nc.alloc_sbuf_tensor("tot_a", [128, 1], F32)
    m1_sb = nc.alloc_sbuf_tensor("m1_sb", [128, NT], F32)
    m2_sb = nc.alloc_sbuf_tensor("m2_sb", [128, NT], F32)
    dg1_slots = nc.alloc_sbuf_tensor("dg1_slots", [128, 8], F32)
    dg2_slots = nc.alloc_sbuf_tensor("dg2_slots", [64, 4], F32)
    bf_rx = [nc.alloc_sbuf_tensor(f"bfrx{r}", [128, PAYB], F32)
             for r in range(3)]
    logs_b = nc.alloc_sbuf_tensor("logs_b", [128, 3 * (NT - ART)], F32)
    lse_b = nc.alloc_sbuf_tensor("lse_b", [128, NT - ART], F32)
    t2m_b = nc.alloc_sbuf_tensor("t2m_b", [128, NT - ART], F32)
    tot = nc.alloc_sbuf_tensor("tot", [128, 1], F32)
    dgr = nc.alloc_sbuf_tensor("dgr", [128, 1], F32)
    t2r = nc.alloc_sbuf_tensor("t2r", [64, 1], F32)
    out_sb = nc.alloc_sbuf_tensor("out_sb", [1, 1], F32)
    totr = nc.alloc_sbuf_tensor("totr", [128, 1], F32)

    with tile.TileContext(nc) as tc:
        with (
            tc.tile_pool(name="res", bufs=1) as res,
            tc.tile_pool(name="dram", bufs=1, space="DRAM") as dram,
        ):
            # ---- resident loads (critical-path first) ----------------------
            p1q_sb = res.tile([128, 8 * R1], F8)
            nc.sync.dma_start(out=p1q_sb[:], in_=p1q_in.ap().bitcast(F8))
            p2q_sb = res.tile([128, 8 * R2], F8)
            nc.sync.dma_start(out=p2q_sb[:], in_=p2q_in.ap().bitcast(F8))
            xq_sb = res.tile([128, 8 * N], F8)
            xq_sb4 = xq_sb[:].rearrange("p (c i n) -> p c i n", c=4, i=2)
            xq_in4 = xq_in.ap().bitcast(F8).rearrange(
                "p (c i n) -> p c i n", c=4, i=2)
            for qt in range(4):
                ts = slice(qt * 1024, (qt + 1) * 1024)
                nc.sync.dma_start(out=xq_sb4[:, :, :, ts],
                                  in_=xq_in4[:, :, :, ts])
            whq_sb = res.tile([128, 8 * VHC], F8)
            for c in range(4):
                cs = slice(c * 2 * VHC, (c + 1) * 2 * VHC)
                nc.sync.dma_start(out=whq_sb[:, cs],
                                  in_=whq_in.ap().bitcast(F8)[:, cs])
            w1q_sb = res.tile([128, 2 * V1C], F8)
            nc.sync.dma_start(out=w1q_sb[:], in_=w1q_in.ap().bitcast(F8))
            w2q_sb = res.tile([64, V2C], F8)
            nc.sync.dma_start(out=w2q_sb[:], in_=w2q_in.ap().bitcast(F8))
            w1sq_sb = res.tile([128, 2 * N], F8)
            w2sq_sb = res.tile([64, N], F8)

            h1q_sb = res.tile([128, 2 * N], F8)
            h2q_sb = res.tile([64, N], F8)
            slots = res.tile([128, NS * NT], F32)
            dsh_slots = res.tile([128, 8], F32)

            # DoubleRow-layout views: [128, c-chunk, i, cols]
            xq_c = [xq_sb[:, c * 2 * N:(c + 1) * 2 * N]
                    .rearrange("p (i n) -> p i n", i=2) for c in range(4)]
            whq_c = [whq_sb[:, c * 2 * VHC:(c + 1) * 2 * VHC]
                     .rearrange("p (i v) -> p i v", i=2) for c in range(4)]
            p1q_c = [p1q_sb[:, c * 2 * R1:(c + 1) * 2 * R1]
                     .rearrange("p (i r) -> p i r", i=2) for c in range(4)]
            p2q_c = [p2q_sb[:, c * 2 * R2:(c + 1) * 2 * R2]
                     .rearrange("p (i r) -> p i r", i=2) for c in range(4)]
            w1q_v = w1q_sb[:].rearrange("p (i v) -> p i v", i=2)
            h1q_v = h1q_sb[:].rearrange("p (i n) -> p i n", i=2)
            w1sq_v = w1sq_sb[:].rearrange("p (i n) -> p i n", i=2)

            # ---- phase A: projections h1 = P1 @ x.T, h2 = P2 @ x.T ---------
            # quarter-pairs share each stationary load across 2 matmuls
            with tc.tile_pool(name="pj", bufs=1, space="PSUM") as pj:
                for qp in range(4):
                    qsl = [slice((qp * 2 + j) * 512, (qp * 2 + j + 1) * 512)
                           for j in range(2)]
                    pas = [pj.tile([128, 512], F32, tag="pa", bufs=2,
                                   name=f"pa{qp}_{j}") for j in range(2)]
                    pbs = [pj.tile([128, 512], F32, tag="pb", bufs=2,
                                   name=f"pb{qp}_{j}") for j in range(2)]
                    pcs = [pj.tile([64, 512], F32, tag="pc", bufs=2,
                                   name=f"pc{qp}_{j}") for j in range(2)]
                    for c in range(4):
                        st = dict(start=(c == 0), stop=(c == 3))
                        for j in range(2):
                            nc.tensor.matmul(pas[j][:],
                                             lhsT=p1q_c[c][:, :, 0:128],
                                             rhs=xq_c[c][:, :, qsl[j]],
                                             perf_mode=DR, **st)
                        for j in range(2):
                            nc.tensor.matmul(pbs[j][:],
                                             lhsT=p1q_c[c][:, :, 128:256],
                                             rhs=xq_c[c][:, :, qsl[j]],
                                             perf_mode=DR, **st)
                        for j in range(2):
                            nc.tensor.matmul(pcs[j][:],
                                             lhsT=p2q_c[c][:, :, 0:64],
                                             rhs=xq_c[c][:, :, qsl[j]],
                                             perf_mode=DR, **st)
                    for j in range(2):
                        nc.scalar.mul(h1q_v[:, 0, qsl[j]], pas[j][:], INV)
                        nc.vector.tensor_scalar_mul(h1q_v[:, 1, qsl[j]],
                                                    pbs[j][:], INV)
                        nc.vector.tensor_scalar_mul(h2q_sb[:, qsl[j]],
                                                    pcs[j][:], INV)

            # dot-product inputs issued late so the phase-A-critical loads
            # (p1q/p2q/xq) get full HBM bandwidth at launch; these are only
            # consumed by DVE jobs from tile 8 onward
            nc.sync.dma_start(out=w1sq_sb[:], in_=w1sq_in.ap().bitcast(F8))
            nc.sync.dma_start(out=w2sq_sb[:], in_=w2sq_in.ap().bitcast(F8))
            nc.sync.dma_start(out=m1_sb[:], in_=m1_in.ap())
            nc.sync.dma_start(out=m2_sb[:], in_=m2_in.ap())

            # ---- phase C: head + tail logits, exp(bf16), per-token sums ----
            # tail target-dot chunks (phase A') and the sharded head-dot
            # stream (phase B) are interleaved at tile boundaries so the DVE
            # work rides under the scalar-engine exp stream.
            dve_jobs = []
            for k in range(8):              # sharded head dot: 8 k-chunks
                dve_jobs.append(("dsh", 0, k))   # first: needed by early AR
            for h in range(2):              # tail1 dot: 8 chunks of 1024
                for qq in range(4):
                    dve_jobs.append(("t1", h, qq))
            for qq in range(4):             # tail2 dot: 4 chunks of 1024
                dve_jobs.append(("t2", 0, qq))

            slots3 = slots[:].rearrange("p (t u) -> p t u", u=NS)
            pay_a = res.tile([128, PAYA], F32)
            paya_dram = dram.tile([128, PAYA], F32)
            reda_dram = dram.tile([128, PAYA], F32)
            if BFLY:
                # memset the butterfly rx buffers NOW so they are
                # initialized long before any peer's remote write can land
                # (peers send only after their phase C)
                for r in range(3):
                    nc.vector.memset(bf_rx[r][:], 0.0)
                bf_rsem = [nc.alloc_semaphore(f"bfr{r}") for r in range(3)]
                bf_lsem = nc.alloc_semaphore("bfl")
                bf_da = nc.alloc_semaphore("bfda")
                tsem = [nc.alloc_semaphore(f"ts{i}") for i in range(5)]
            else:
                payb_dram = dram.tile([128, PAYB], F32)
                redb_dram = dram.tile([128, PAYB], F32)
                red_b = res.tile([128, PAYB], F32)
            logs_a = res.tile([128, 3 * ART], F32)
            lse_a = res.tile([128, ART], F32)
            t2m_a = res.tile([128, ART], F32)

            with (
                tc.tile_pool(name="pp", bufs=4, space="PSUM") as pp,
                tc.tile_pool(name="eb", bufs=6) as ebp,
                tc.tile_pool(name="ac", bufs=2) as acp,
                tc.tile_pool(name="ds", bufs=2) as dsp,
            ):
                job_i = 0
                for nt in range(NT):
                    nts = slice(nt * 128, (nt + 1) * 128)
                    ebs = []
                    for u, (cl, base, w) in enumerate(UNITS):
                        pu = pp.tile([128, 1024], F32, tag="pu")
                        # lhsT-outer loops: consecutive matmuls reuse the
                        # stationary operand so LDWEIGHTS amortizes/hides
                        if cl == "H":
                            for c in range(4):
                                for off in range(0, w, 512):
                                    cw = min(512, w - off)
                                    nc.tensor.matmul(
                                        pu[:, off:off + cw],
                                        lhsT=xq_c[c][:, :, nts],
                                        rhs=whq_c[c][:, :, base + off:
                                                     base + off + cw],
                                        perf_mode=DR,
                                        start=(c == 0), stop=(c == 3))
                        else:
                            for off in range(0, w, 512):
                                cw = min(512, w - off)
                                vs = slice(base + off, base + off + cw)
                                if cl == "T1":
                                    nc.tensor.matmul(
                                        pu[:, off:off + cw],
                                        lhsT=h1q_v[:, :, nts],
                                        rhs=w1q_v[:, :, vs], perf_mode=DR,
                                        start=True, stop=True)
                                else:
                                    nc.tensor.matmul(
                                        pu[:, off:off + cw],
                                        lhsT=h2q_sb[:, nts],
                                        rhs=w2q_sb[:, vs],
                                        start=True, stop=True)
                        if cl == "T2":
                            # in-place exp + ACT accumulator: keeps ~20% of
                            # the sum work off the (busier) vector engine
                            col = nt * NS + 4 + (u - 6)
                            nc.scalar.activation(
                                pu[:, 0:w], pu[:, 0:w], AF.Exp, scale=INV,
                                accum_out=slots[:, col:col + 1])
                        else:
                            eb = ebp.tile([128, 1024], BF16, tag="eb")
                            nc.scalar.activation(eb[:, 0:w], pu[:, 0:w],
                                                 AF.Exp, scale=INV)
                            ebs.append(eb)
                        # pair-fold on DVE (2x bf16 tensor_tensor), then a
                        # short 1x reduce: H units 0+1 -> slot0, unit2 ->
                        # slot1; T1 units -> slots 2,3
                        if u in (1, 4):
                            sc = nt * NS + (0 if u == 1 else 2)
                            acc = acp.tile([128, 1024], BF16, tag="acc")
                            nc.vector.tensor_add(acc[:], ebs[-2][:],
                                                 ebs[-1][:])
                            acc2 = acp.tile([128, 512], BF16, tag="acc2")
                            nc.vector.tensor_add(acc2[:], acc[:, 0:512],
                                                 acc[:, 512:1024])
                            nc.vector.reduce_sum(slots[:, sc:sc + 1],
                                                 acc2[:], axis=AXX)
                        elif u in (2, 5):
                            sc = nt * NS + (1 if u == 2 else 3)
                            nc.vector.reduce_sum(slots[:, sc:sc + 1],
                                                 ebs[-1][:, 0:w], axis=AXX)
                    # interleave 2 DVE dot jobs per tile starting at tile 8
                    if nt >= 8:
                        for _ in range(2):
                            if job_i < len(dve_jobs):
                                _emit_dot(nc, dsp, dve_jobs[job_i], h1q_v,
                                          w1sq_v, h2q_sb, w2sq_sb, xTc_in,
                                          wselT_in, dg1_slots, dg2_slots,
                                          dsh_slots)
                                job_i += 1
                    if nt == ART - 1:
                        # early AllReduce chunk: tiles 0..ART-1 + dsh.
                        # Overlaps the remaining compute and absorbs
                        # inter-core launch skew.
                        nc.vector.reduce_sum(pay_a[:, 0:ART],
                                             slots3[:, 0:ART, 0:2], axis=AXX)
                        nc.vector.reduce_sum(pay_a[:, ART:2 * ART],
                                             slots3[:, 0:ART, 2:4], axis=AXX)
                        nc.vector.reduce_sum(pay_a[:, 2 * ART:3 * ART],
                                             slots3[:, 0:ART, 4:6], axis=AXX)
                        nc.vector.reduce_sum(pay_a[:, 3 * ART:PAYA],
                                             dsh_slots[:], axis=AXX)
                        nc.sync.dma_start(out=paya_dram[:], in_=pay_a[:])
                        nc.gpsimd.collective_compute(
                            "AllReduce", ALU.add,
                            replica_groups=[list(range(NCORES))],
                            ins=[paya_dram.opt()], outs=[reda_dram.opt()])
                        nc.sync.dma_start(out=red_a[:], in_=reda_dram[:])
                    if nt == NT - 3:
                        # early chunk of the NLL finish, overlapped with the
                        # last tiles (red_a has landed by now)
                        nc.vector.tensor_scalar_add(
                            red_a[:, 0:ART], red_a[:, 0:ART], float(-PAD_H))
                        nc.vector.tensor_scalar_add(
                            red_a[:, 2 * ART:3 * ART],
                            red_a[:, 2 * ART:3 * ART], float(-PAD_2))
                        nc.scalar.activation(logs_a[:], red_a[:, 0:3 * ART],
                                             AF.Ln)
                        nc.vector.tensor_mul(lse_a[:],
                                             logs_a[:, ART:2 * ART],
                                             m1_sb[:, 0:ART])
                        nc.vector.tensor_mul(t2m_a[:],
                                             logs_a[:, 2 * ART:3 * ART],
                                             m2_sb[:, 0:ART])
                        nc.vector.tensor_add(lse_a[:], lse_a[:],
                                             logs_a[:, 0:ART])
                        nc.vector.tensor_add(lse_a[:], lse_a[:], t2m_a[:])
                        nc.vector.reduce_sum(tot_a[:], lse_a[:], axis=AXX)
                        nc.vector.reduce_sum(dgr[:], dg1_slots[:], axis=AXX)
                        nc.vector.tensor_scalar_mul(dgr[:], dgr[:], INV)
                        nc.vector.reduce_sum(t2r[:], dg2_slots[:], axis=AXX)
                        nc.vector.tensor_scalar_mul(t2r[:], t2r[:], INV)
                while job_i < len(dve_jobs):
                    _emit_dot(nc, dsp, dve_jobs[job_i], h1q_v, w1sq_v,
                              h2q_sb, w2sq_sb, xTc_in, wselT_in,
                              dg1_slots, dg2_slots, dsh_slots)
                    job_i += 1

            # ---- phase D: fold remaining tiles, late AllReduce chunk -------
            NTB = NT - ART
            nc.vector.reduce_sum(pay_b[:, 0:NTB],
                                 slots3[:, ART:NT, 0:2], axis=AXX)
            nc.vector.reduce_sum(pay_b[:, NTB:2 * NTB],
                                 slots3[:, ART:NT, 2:4], axis=AXX)
            nc.vector.reduce_sum(pay_b[:, 2 * NTB:3 * NTB],
                                 slots3[:, ART:NT, 4:6], axis=AXX)
            if not BFLY:
                nc.sync.dma_start(out=payb_dram[:], in_=pay_b[:])
                nc.gpsimd.collective_compute(
                    "AllReduce", ALU.add,
                    replica_groups=[list(range(NCORES))],
                    ins=[payb_dram.opt()], outs=[redb_dram.opt()])
                nc.sync.dma_start(out=red_b[:], in_=redb_dram[:])
                _phase_e_tail(nc, red_b, logs_b, lse_b, t2m_b, tot, dgr,
                              t2r, out_sb, m1_sb, m2_sb, tot_a, red_a,
                              dg1_slots, dg2_slots, out_ext, sems=None, totr=totr)

    if BFLY:
        # The butterfly exchange + NLL finish live OUTSIDE the TileContext:
        # the tile scheduler's no-exec sim cannot model SWDGE/remote
        # semaphore updates and would declare a deadlock. Per-engine program
        # order (continuous across blocks) plus explicit semaphores provide
        # all ordering here.
        NTB = NT - ART
        with nc.Block("bfly_tail"):
            # XOR-butterfly AllReduce over remote SBUF DMA: 3 rounds of
            # pairwise exchange (partner = me ^ 1, ^2, ^4); each round
            # pay_b += partner's partial. Replaces a ~26us collective.
            RD = [[None] * 8 for _ in range(3)]
            RD[0][0] = (0, 1)
            RD[1][0] = (0, 2)
            RD[2][4] = (0, 4)   # bit-2 dest must sit in a D2D slot
            for r in range(3):
                bc = nc.gpsimd.remote_dma_broadcast(
                    out_ap=bf_rx[r][:], in_ap=pay_b[:],
                    remote_sem=bf_rsem[r], local_sem=bf_lsem,
                    rdests=RD[r])
                if r > 0:
                    bc._wait_ge(bf_da, r)
                nc.gpsimd.trigger_dma(1)
                nc.vector.wait_ge(bf_rsem[r], 2)
                nc.vector.wait_ge(bf_lsem, 16 * (r + 1))
                add = nc.vector.tensor_add(pay_b[:], pay_b[:], bf_rx[r][:])
                add.then_inc(bf_da, 1)
            _phase_e_tail(nc, pay_b, logs_b, lse_b, t2m_b, tot, dgr, t2r,
                          out_sb, m1_sb, m2_sb, tot_a, red_a, dg1_slots,
                          dg2_slots, out_ext, sems=tsem, totr=totr)

    nc.compile()
    return nc


def _phase_e_tail(nc, red_b, logs_b, lse_b, t2m_b, tot, dgr, t2r, out_sb,
                  m1_sb, m2_sb, tot_a, red_a, dg1_slots, dg2_slots, out_ext,
                  sems, totr=None):
    """Finish the NLL from the late-chunk sums. With sems (raw post-tile
    block) explicit semaphores sequence the DVE->ACT->DVE->Pool->DMA chain;
    without (inside TileContext) tile data-deps handle it."""
    NTB = NT - ART
    nc.vector.tensor_scalar_add(red_b[:, 0:NTB], red_b[:, 0:NTB],
                                float(-PAD_H))
    a2 = nc.vector.tensor_scalar_add(red_b[:, 2 * NTB:3 * NTB],
                                     red_b[:, 2 * NTB:3 * NTB],
                                     float(-PAD_2))
    ln = nc.scalar.activation(logs_b[:], red_b[:, 0:3 * NTB], AF.Ln)
    if sems:
        a2.then_inc(sems[0], 1)
        ln._wait_ge(sems[0], 1)
        ln.then_inc(sems[1], 1)
    mu = nc.vector.tensor_mul(lse_b[:], logs_b[:, NTB:2 * NTB],
                              m1_sb[:, ART:NT])
    if sems:
        mu._wait_ge(sems[1], 1)
    nc.vector.tensor_mul(t2m_b[:], logs_b[:, 2 * NTB:3 * NTB],
                         m2_sb[:, ART:NT])
    nc.vector.tensor_add(lse_b[:], lse_b[:], logs_b[:, 0:NTB])
    nc.vector.tensor_add(lse_b[:], lse_b[:], t2m_b[:])
    nc.vector.reduce_sum(tot[:], lse_b[:], axis=AXX)
    nc.vector.tensor_add(tot[:], tot[:], tot_a[:])
    nc.vector.tensor_sub(tot[:], tot[:], red_a[:, 3 * ART:PAYA])
    nc.vector.tensor_sub(tot[:], tot[:], dgr[:])
    last = nc.vector.tensor_sub(tot[:64, :], tot[:64, :], t2r[:])
    gr = nc.gpsimd.partition_all_reduce(totr[:], tot[:], 128,
                                        bass_isa.ReduceOp.add)
    if sems:
        last.then_inc(sems[2], 1)
        gr._wait_ge(sems[2], 1)
        gr.then_inc(sems[3], 1)
    od = nc.sync.dma_start(out=out_ext.ap(), in_=totr[0:1, 0:1])
    if sems:
        od._wait_ge(sems[3], 1)
        od.then_inc(sems[4], 16)
        nc.sync.wait_ge(sems[4], 16)


def _emit_dot(nc, dsp, job, h1q_v, w1sq_v, h2q_sb, w2sq_sb, xTc_in,
              wselT_in, dg1_slots, dg2_slots, dsh_slots):
    kind, h, k = job
    if kind == "t1":
        ks = slice(k * 1024, (k + 1) * 1024)
        scr = dsp.tile([128, 1024], BF16, tag="scr")
        nc.vector.tensor_mul(scr[:], h1q_v[:, h, ks], w1sq_v[:, h, ks])
        col = h * 4 + k
        nc.vector.reduce_sum(dg1_slots[:, col:col + 1], scr[:], axis=AXX)
    elif kind == "t2":
        ks = slice(k * 1024, (k + 1) * 1024)
        scr = dsp.tile([64, 1024], BF16, tag="scr2")
        nc.vector.tensor_mul(scr[:], h2q_sb[:, ks], w2sq_sb[:, ks])
        nc.vector.reduce_sum(dg2_slots[:, k:k + 1], scr[:], axis=AXX)
    else:  # dsh
        xc = dsp.tile([128, NSH], F32, tag="xc")
        nc.sync.dma_start(out=xc[:], in_=xTc_in.ap()[k * 128:(k + 1) * 128, :])
        wc = dsp.tile([128, NSH], F32, tag="wc")
        nc.sync.dma_start(out=wc[:],
                          in_=wselT_in.ap()[k * 128:(k + 1) * 128, :])
        scr = dsp.tile([128, NSH], BF16, tag="scr3")
        nc.vector.tensor_mul(scr[:], xc[:], wc[:])
        nc.vector.reduce_sum(dsh_slots[:, k:k + 1], scr[:], axis=AXX)


_NC = None


def _get_nc():
    global _NC
    if _NC is None:
        _NC = _build()
    return _NC


def _dr_layout_d(mT):
    """[D, cols] -> DoubleRow layout [128, 4*2*cols] for contraction over D."""
    cols = mT.shape[1]
    return np.ascontiguousarray(
        mT.reshape(4, 2, 128, cols).transpose(2, 0, 1, 3).reshape(128, -1))


def _dr_layout_r(mT):
    """[256, cols] -> DoubleRow layout [128, 2*cols] for contraction over R1."""
    cols = mT.shape[1]
    return np.ascontiguousarray(
        mT.reshape(2, 128, cols).transpose(1, 0, 2).reshape(128, -1))


def _prep_inputs(inputs):
    x = np.ascontiguousarray(inputs["x"], dtype=np.float32)
    target = np.asarray(inputs["target"]).astype(np.int64)
    W_head = np.asarray(inputs["W_head"], dtype=np.float32)
    W_cluster = np.asarray(inputs["W_cluster"], dtype=np.float32)
    P1 = np.asarray(inputs["P1"], dtype=np.float32)
    W1 = np.asarray(inputs["W1"], dtype=np.float32)
    P2 = np.asarray(inputs["P2"], dtype=np.float32)
    W2 = np.asarray(inputs["W2"], dtype=np.float32)

    W_ext = np.concatenate([W_head, W_cluster], axis=0)          # [20002, D]
    mask1 = (target >= C0) & (target < C1)
    mask2 = target >= C1
    cidx = np.where(target < C0, target,
                    np.where(mask1, C0, C0 + 1)).astype(np.int64)
    W1p = np.concatenate([WS * W1, np.zeros((1, R1), np.float32)], axis=0)
    W2p = np.concatenate([WS * W2, np.zeros((1, R2), np.float32)], axis=0)
    j1 = np.where(mask1, target - C0, C1 - C0).astype(np.int64)
    j2 = np.where(mask2, target - C1, C2 - C1).astype(np.int64)

    xT = np.ascontiguousarray(x.T)                               # [D, N]
    xq = _dr_layout_d(xT.astype(NP_F8)).view(np.uint8)

    Whfull = np.zeros((NCORES * VHC, D), np.float32)
    Whfull[:VH] = WS * W_ext
    W1full = np.zeros((NCORES * V1C, R1), np.float32)
    W1full[:C1 - C0] = WS * W1
    W2full = np.zeros((NCORES * V2C, R2), np.float32)
    W2full[:C2 - C1] = WS * W2

    p1q = _dr_layout_d((WS * P1).T.astype(NP_F8)).view(np.uint8)
    p2q = _dr_layout_d((WS * P2).T.astype(NP_F8)).view(np.uint8)
    w1sq = _dr_layout_r(W1p[j1].T.astype(NP_F8)).view(np.uint8)
    w2sq = np.ascontiguousarray(W2p[j2].T.astype(NP_F8)).view(np.uint8)
    wselT = np.ascontiguousarray(W_ext[cidx].T)                  # [D, N] f32
    m1 = np.ascontiguousarray(mask1.astype(np.float32).reshape(NT, 128).T)
    m2 = np.ascontiguousarray(mask2.astype(np.float32).reshape(NT, 128).T)

    in_maps = []
    for i in range(NCORES):
        whq = _dr_layout_d(
            np.ascontiguousarray(
                Whfull[i * VHC:(i + 1) * VHC].T).astype(NP_F8)).view(np.uint8)
        w1q = _dr_layout_r(
            np.ascontiguousarray(
                W1full[i * V1C:(i + 1) * V1C].T).astype(NP_F8)).view(np.uint8)
        w2q = np.ascontiguousarray(
            W2full[i * V2C:(i + 1) * V2C].T.astype(NP_F8)).view(np.uint8)
        in_maps.append({
            "xq": xq,
            "whq": whq,
            "w1q": w1q,
            "w2q": w2q,
            "p1q": p1q,
            "p2q": p2q,
            "w1sq": w1sq,
            "w2sq": w2sq,
            "xTc": np.ascontiguousarray(xT[:, i * NSH:(i + 1) * NSH]),
            "wselT": np.ascontiguousarray(wselT[:, i * NSH:(i + 1) * NSH]),
            "m1": m1,
            "m2": m2,
        })
    return in_maps


def kernel(**inputs):
    in_maps = _prep_inputs(inputs)
    nc = _get_nc()
    trace = bool(int(os.environ.get("KERNEL_TRACE", "0")))
    if trace:
        _install_ntff_hook()
    res = run_bass_kernel_spmd(nc, in_maps, core_ids=list(range(NCORES)),
                               trace=trace)
    global LAST_EXEC_NS
    LAST_EXEC_NS = res.exec_time_ns
    val = np.float32(res.results[0]["out"][0, 0])
    return np.asarray(val, dtype=np.float32)


def _install_ntff_hook():
    """Shim antenv.axon_hooks so trace=True can capture NTFF profiles."""
    import types
    import antenv
    if hasattr(antenv, "axon_hooks"):
        return
    hooks = types.ModuleType("antenv.axon_hooks")
    holder = [None]
    hooks.set_axon_ntff_profile_hook = lambda h: holder.__setitem__(0, h)
    hooks.get_axon_ntff_profile_hook = lambda: holder[0]
    sys.modules["antenv.axon_hooks"] = hooks
    antenv.axon_hooks = hooks
    try:
        from trn_agent_boot.trn_boot import _ntff_profile_via_ctypes
        hooks.set_axon_ntff_profile_hook(
            _ntff_profile_via_ctypes("/opt/axon/libaxon_pjrt.so"))
    except Exception:
        pass
